# revision 6
# baseline (speedup 1.0000x reference)
"""Multi-head self-attention (B=2, S=2048, E=1024, H=16, D=64) on 8 TRN2 cores.

Sharding: tensor-parallel over (batch, head-group): core c handles batch c//4
and heads [4*(c%4), 4*(c%4)+4). Each core computes its heads' attention output
projected through its slice of Wo; the host sums the 4 partial outputs per
batch and adds the constant bias row (bv @ Wo + bo).

Device-side math (per core, transposed formulation so no transposes needed):
  QT = Wq_c^T @ x^T + bq_c        [256, S]   (bias bk dropped: softmax-invariant)
  KT = Wk_c^T @ x^T               [256, S]
  V  = x @ Wv_c                   [S, 256]   (bias bv folded into host bias row)
  S^T tile = K @ Q^T              (PE, per 128-k-token x 1024-q tile)
  P^T = exp(S^T / 8)              (ACT, no max subtraction: scores ~ N(0,1))
  O^T aug = [V | 1]^T @ P^T       (PE, accumulated over k tiles; row 64 = sum)
  O^T = O^T aug[0:64] / row 64    (recip + PE broadcast + DVE mul)
  Y = O @ Wo_c                    [S, 1024]  fp32 partial out
"""

import numpy as np
import ml_dtypes

import concourse.bass as bass
import concourse.bacc as bacc
import concourse.tile as tile
from concourse import mybir
from concourse.bass_utils import run_bass_kernel_spmd

B, S, E = 2, 2048, 1024
H, D = 16, 64
NCORES = 8
HPC = 4                 # heads per core
EH = HPC * D            # 256: per-core head width
P = 128
EC = E // P             # 8 E-chunks of 128
MC = EH // P            # 2 Eh-chunks of 128
NT = S // P             # 16 token tiles of 128
QH = 1024               # q-chunk processed per attention unit
NQH = S // QH           # 2
SCALE = 1.0 / float(np.sqrt(D))

DT = mybir.dt.bfloat16
NP_DT = ml_dtypes.bfloat16
F32 = mybir.dt.float32
F32R = mybir.dt.float32r

AF = mybir.ActivationFunctionType


def build_nc():
    nc = bacc.Bacc(
        "TRN2", target_bir_lowering=False, debug=False, enable_asserts=False
    )
    xT = nc.dram_tensor("xT", [E, S], DT, kind="ExternalInput").ap()
    wq = nc.dram_tensor("wq", [E, EH], DT, kind="ExternalInput").ap()
    wk = nc.dram_tensor("wk", [E, EH], DT, kind="ExternalInput").ap()
    wv = nc.dram_tensor("wv", [E, EH], DT, kind="ExternalInput").ap()
    wo = nc.dram_tensor("wo", [EH, E], DT, kind="ExternalInput").ap()
    bq = nc.dram_tensor("bq", [EH], F32, kind="ExternalInput").ap()
    y = nc.dram_tensor("y", [S, E], F32, kind="ExternalOutput").ap()

    with tile.TileContext(nc) as tc:
        with (
            tc.tile_pool(name="consts", bufs=1) as consts,
            tc.tile_pool(name="work", bufs=4) as work,
            tc.tile_pool(name="norm", bufs=2) as norm,
            tc.tile_pool(name="outsb", bufs=2) as outsb,
            tc.tile_pool(name="psA", bufs=2, space="PSUM") as psA,
            tc.tile_pool(name="psO", bufs=2, space="PSUM") as psO,
        ):
            # ---- constant loads ----
            xT_sb = consts.tile([P, EC, S], DT)
            nc.sync.dma_start(out=xT_sb, in_=xT.rearrange("(c p) s -> p c s", p=P))
            wq_sb = consts.tile([P, EC, EH], DT)
            nc.sync.dma_start(out=wq_sb, in_=wq.rearrange("(c p) n -> p c n", p=P))
            wk_sb = consts.tile([P, EC, EH], DT)
            nc.sync.dma_start(out=wk_sb, in_=wk.rearrange("(c p) n -> p c n", p=P))
            wv_sb = consts.tile([P, EC, EH], DT)
            nc.sync.dma_start(out=wv_sb, in_=wv.rearrange("(c p) n -> p c n", p=P))
            wo_sb = consts.tile([P, MC, E], DT)
            nc.sync.dma_start(out=wo_sb, in_=wo.rearrange("(m p) n -> p m n", p=P))
            bq_sb = consts.tile([P, MC], F32)
            nc.sync.dma_start(out=bq_sb, in_=bq.rearrange("(m p) -> p m", p=P))
            ones64 = consts.tile([1, 64], F32)
            nc.vector.memset(ones64, 1.0)

            QT_sb = consts.tile([P, MC, S], DT)
            KT_sb = consts.tile([P, MC, S], DT)
            V_sb = consts.tile([P, NT, HPC, D + 1], DT)
            OT_sb = consts.tile([P, MC, S], DT)
            nc.vector.memset(V_sb[:, :, :, D : D + 1], 1.0)

            # ---- QKV projections ----
            for mc in range(MC):
                for t4 in range(S // 512):
                    sl = bass.ts(t4, 512)
                    for w_sb, dst, is_q in (
                        (wq_sb, QT_sb, True),
                        (wk_sb, KT_sb, False),
                    ):
                        ps = psA.tile([P, 512], F32, tag="big")
                        for ec in range(EC):
                            nc.tensor.matmul(
                                ps,
                                lhsT=w_sb[:, ec, mc * P : (mc + 1) * P],
                                rhs=xT_sb[:, ec, sl],
                                start=(ec == 0),
                                stop=(ec == EC - 1),
                            )
                        if is_q:
                            nc.scalar.activation(
                                out=dst[:, mc, sl],
                                in_=ps,
                                func=AF.Identity,
                                bias=bq_sb[:, mc : mc + 1],
                                scale=1.0,
                            )
                        else:
                            nc.scalar.copy(out=dst[:, mc, sl], in_=ps)

            for t in range(NT):
                ps = psA.tile([P, EH], F32, tag="big")
                for ec in range(EC):
                    nc.tensor.matmul(
                        ps,
                        lhsT=xT_sb[:, ec, bass.ts(t, P)],
                        rhs=wv_sb[:, ec, :],
                        start=(ec == 0),
                        stop=(ec == EC - 1),
                    )
                for h in range(HPC):
                    nc.vector.tensor_copy(
                        out=V_sb[:, t, h, 0:D], in_=ps[:, h * D : (h + 1) * D]
                    )

            # ---- attention: head pair hp = heads (2hp, 2hp+1), q chunk qh ----
            for hp in range(MC):
                for iq in range(NQH):
                    q0 = iq * QH
                    O_pair = [
                        psO.tile([D + 1, QH], F32, tag="acc", name=f"O{hp}{iq}a"),
                        psO.tile([D + 1, QH], F32, tag="acc", name=f"O{hp}{iq}b"),
                    ]
                    for kt in range(NT):
                        ST_pair = [
                            psA.tile([P, QH], F32, tag="big", name=f"ST{kt}a"),
                            psA.tile([P, QH], F32, tag="big", name=f"ST{kt}b"),
                        ]
                        # scores^T: row-group packed pair (bases 0 / 64)
                        for qs in range(QH // 512):
                            for i, base in ((0, 0), (1, 64)):
                                nc.tensor.matmul(
                                    ST_pair[i][:, bass.ts(qs, 512)],
                                    lhsT=KT_sb[base : base + 64, hp, bass.ts(kt, P)],
                                    rhs=QT_sb[
                                        base : base + 64,
                                        hp,
                                        q0 + qs * 512 : q0 + (qs + 1) * 512,
                                    ],
                                    start=True,
                                    stop=True,
                                )
                        for i in range(2):
                            h_local = 2 * hp + i
                            PT = work.tile([P, QH], DT, tag="pt")
                            nc.scalar.activation(
                                out=PT, in_=ST_pair[i], func=AF.Exp, scale=SCALE
                            )
                            for qs in range(QH // 512):
                                nc.tensor.matmul(
                                    O_pair[i][:, bass.ts(qs, 512)],
                                    lhsT=V_sb[:, kt, h_local, :],
                                    rhs=PT[:, bass.ts(qs, 512)],
                                    start=(kt == 0),
                                    stop=(kt == NT - 1),
                                )
                    # normalize + evacuate O^T
                    for i, base in ((0, 0), (1, 64)):
                        ex = norm.tile([1, QH], F32, tag="ex")
                        nc.vector.tensor_copy(out=ex, in_=O_pair[i][D : D + 1, :])
                        rc = norm.tile([1, QH], F32, tag="rc")
                        nc.vector.reciprocal(out=rc, in_=ex)
                        bc = norm.tile([64, QH], F32, tag="bc")
                        for qs in range(QH // 512):
                            psbc = psA.tile([64, 512], F32, tag="big")
                            nc.tensor.matmul(
                                psbc,
                                lhsT=ones64,
                                rhs=rc[:, bass.ts(qs, 512)],
                                start=True,
                                stop=True,
                            )
                            nc.vector.tensor_copy(
                                out=bc[:, bass.ts(qs, 512)], in_=psbc
                            )
                        nc.vector.tensor_mul(
                            out=OT_sb[base : base + 64, hp, q0 : q0 + QH],
                            in0=O_pair[i][0:D, :],
                            in1=bc,
                        )

            # ---- output projection Y = O @ Wo_c ----
            y_r = y.rearrange("(t p) n -> t p n", p=P)
            for t in range(NT):
                psY = psA.tile([P, E], F32, tag="big")
                for n2 in range(E // 512):
                    for mc in range(MC):
                        nc.tensor.matmul(
                            psY[:, bass.ts(n2, 512)],
                            lhsT=OT_sb[:, mc, bass.ts(t, P)],
                            rhs=wo_sb[:, mc, bass.ts(n2, 512)],
                            start=(mc == 0),
                            stop=(mc == MC - 1),
                        )
                y_sb = outsb.tile([P, E], F32, tag="ysb")
                nc.vector.tensor_copy(out=y_sb, in_=psY)
                nc.sync.dma_start(out=y_r[t], in_=y_sb)

    nc.compile()
    return nc


_NC_CACHE = {}


def get_nc():
    if "nc" not in _NC_CACHE:
        _NC_CACHE["nc"] = build_nc()
    return _NC_CACHE["nc"]


def make_in_maps(x, Wq, bq, Wk, Wv, Wo):
    xT_by_batch = [
        np.ascontiguousarray(x[b].T).astype(NP_DT) for b in range(B)
    ]
    in_maps = []
    for c in range(NCORES):
        b, hg = divmod(c, NCORES // B)
        hs = slice(hg * EH, (hg + 1) * EH)
        in_maps.append(
            {
                "xT": xT_by_batch[b],
                "wq": np.ascontiguousarray(Wq[:, hs]).astype(NP_DT),
                "wk": np.ascontiguousarray(Wk[:, hs]).astype(NP_DT),
                "wv": np.ascontiguousarray(Wv[:, hs]).astype(NP_DT),
                "wo": np.ascontiguousarray(Wo[hs, :]).astype(NP_DT),
                "bq": np.ascontiguousarray(bq[hs]).astype(np.float32),
            }
        )
    return in_maps


def gather_out(results, bv, Wo, bo):
    bias_row = (
        bv.astype(np.float64) @ Wo.astype(np.float64) + bo.astype(np.float64)
    ).astype(np.float32)
    out = np.empty((B, S, E), np.float32)
    gpb = NCORES // B
    for b in range(B):
        acc = results[gpb * b]["y"].copy()
        for i in range(1, gpb):
            acc += results[gpb * b + i]["y"]
        out[b] = acc + bias_row
    return out


def kernel(x, Wq, bq, Wk, bk, Wv, bv, Wo, bo, **_):
    x = np.asarray(x, np.float32)
    nc = get_nc()
    in_maps = make_in_maps(
        x,
        np.asarray(Wq, np.float32),
        np.asarray(bq, np.float32),
        np.asarray(Wk, np.float32),
        np.asarray(Wv, np.float32),
        np.asarray(Wo, np.float32),
    )
    res = run_bass_kernel_spmd(nc, in_maps, list(range(NCORES)))
    return gather_out(
        res.results, np.asarray(bv, np.float32), np.asarray(Wo, np.float32),
        np.asarray(bo, np.float32)
    )


# revision 11
# speedup vs baseline: 1.1777x; 1.1777x over previous
"""Multi-head self-attention (B=2, S=2048, E=1024, H=16, D=64) on 8 TRN2 cores.

Sharding: tensor-parallel over (batch, head-group): core c handles batch c//4
and heads [4*(c%4), 4*(c%4)+4). Each core computes its heads' attention output
projected through its slice of Wo; the host sums the 4 partial outputs per
batch and adds the constant bias row (bv @ Wo + bo).

Device-side math (per core, transposed formulation so no transposes needed):
  QT = Wq_c^T @ x^T + bq_c        [256, S]   (bias bk dropped: softmax-invariant)
  KT = Wk_c^T @ x^T               [256, S]
  V  = x @ Wv_c                   [S, 256]   (bias bv folded into host bias row)
  S^T tile = K @ Q^T              (PE, per 128-k-token x 1024-q tile)
  P^T = exp(S^T / 8)              (ACT, no max subtraction: scores ~ N(0,1))
  O^T aug = [V | 1]^T @ P^T       (PE, accumulated over k tiles; row 64 = sum)
  O^T = O^T aug[0:64] / row 64    (recip + PE broadcast + DVE mul)
  Y = O @ Wo_c                    [S, 1024]  fp32 partial out
"""

import numpy as np
import ml_dtypes

import concourse.bass as bass
import concourse.bacc as bacc
import concourse.tile as tile
from concourse import mybir
from concourse.bass_utils import run_bass_kernel_spmd

B, S, E = 2, 2048, 1024
H, D = 16, 64
NCORES = 8
HPC = 4                 # heads per core
EH = HPC * D            # 256: per-core head width
P = 128
EC = E // P             # 8 E-chunks of 128
MC = EH // P            # 2 Eh-chunks of 128
NT = S // P             # 16 token tiles of 128
QH = 1024               # q-chunk processed per attention unit
NQH = S // QH           # 2
SCALE = 1.0 / float(np.sqrt(D))

DT = mybir.dt.bfloat16
NP_DT = ml_dtypes.bfloat16
F32 = mybir.dt.float32
F32R = mybir.dt.float32r

AF = mybir.ActivationFunctionType


def build_nc():
    nc = bacc.Bacc(
        "TRN2", target_bir_lowering=False, debug=False, enable_asserts=False
    )
    xT = nc.dram_tensor("xT", [E, S], DT, kind="ExternalInput").ap()
    wq = nc.dram_tensor("wq", [E, EH], DT, kind="ExternalInput").ap()
    wk = nc.dram_tensor("wk", [E, EH], DT, kind="ExternalInput").ap()
    wv = nc.dram_tensor("wv", [E, EH], DT, kind="ExternalInput").ap()
    wo = nc.dram_tensor("wo", [EH, E], DT, kind="ExternalInput").ap()
    bq = nc.dram_tensor("bq", [EH], F32, kind="ExternalInput").ap()
    y = nc.dram_tensor("y", [S, E], F32, kind="ExternalOutput").ap()

    with tile.TileContext(nc) as tc:
        with (
            tc.tile_pool(name="consts", bufs=1) as consts,
            tc.tile_pool(name="work", bufs=4) as work,
            tc.tile_pool(name="norm", bufs=2) as norm,
            tc.tile_pool(name="outsb", bufs=2) as outsb,
            tc.tile_pool(name="psA", bufs=2, space="PSUM") as psA,
            tc.tile_pool(name="psO", bufs=2, space="PSUM") as psO,
            tc.tile_pool(name="dram", bufs=2, space="DRAM") as dram,
        ):
            # ---- constant loads ----
            xT_sb = consts.tile([P, EC, S], DT)
            nc.sync.dma_start(out=xT_sb, in_=xT.rearrange("(c p) s -> p c s", p=P))
            wq_sb = consts.tile([P, EC, EH], DT)
            nc.sync.dma_start(out=wq_sb, in_=wq.rearrange("(c p) n -> p c n", p=P))
            wk_sb = consts.tile([P, EC, EH], DT)
            nc.sync.dma_start(out=wk_sb, in_=wk.rearrange("(c p) n -> p c n", p=P))
            wv_sb = consts.tile([P, EC, EH], DT)
            nc.sync.dma_start(out=wv_sb, in_=wv.rearrange("(c p) n -> p c n", p=P))
            wo_sb = consts.tile([P, MC, E], DT)
            nc.sync.dma_start(out=wo_sb, in_=wo.rearrange("(m p) n -> p m n", p=P))
            bq_sb = consts.tile([P, MC], F32)
            nc.sync.dma_start(out=bq_sb, in_=bq.rearrange("(m p) -> p m", p=P))


            QT_sb = consts.tile([P, MC, S], DT)
            KT_sb = consts.tile([P, MC, S], DT)
            V_sb = consts.tile([P, NT, HPC, D + 1], DT)
            OT_sb = consts.tile([P, MC, S], DT)
            nc.vector.memset(V_sb[:, :, :, D : D + 1], 1.0)

            # ---- QKV projections ----
            for mc in range(MC):
                for t4 in range(S // 512):
                    sl = bass.ts(t4, 512)
                    for w_sb, dst, is_q in (
                        (wq_sb, QT_sb, True),
                        (wk_sb, KT_sb, False),
                    ):
                        ps = psA.tile([P, 512], F32, tag="big")
                        for ec in range(EC):
                            nc.tensor.matmul(
                                ps,
                                lhsT=w_sb[:, ec, mc * P : (mc + 1) * P],
                                rhs=xT_sb[:, ec, sl],
                                start=(ec == 0),
                                stop=(ec == EC - 1),
                            )
                        if is_q:
                            nc.scalar.activation(
                                out=dst[:, mc, sl],
                                in_=ps,
                                func=AF.Identity,
                                bias=bq_sb[:, mc : mc + 1],
                                scale=1.0,
                            )
                        else:
                            nc.scalar.copy(out=dst[:, mc, sl], in_=ps)

            for t in range(NT):
                ps = psA.tile([P, EH], F32, tag="big")
                for ec in range(EC):
                    nc.tensor.matmul(
                        ps,
                        lhsT=xT_sb[:, ec, bass.ts(t, P)],
                        rhs=wv_sb[:, ec, :],
                        start=(ec == 0),
                        stop=(ec == EC - 1),
                    )
                for h in range(HPC):
                    nc.vector.tensor_copy(
                        out=V_sb[:, t, h, 0:D], in_=ps[:, h * D : (h + 1) * D]
                    )

            # ---- attention + output projection, pipelined per q-chunk iq ----
            # Per (hp, iq) unit: scores^T -> exp -> [V|1]^T @ P^T, then fast
            # psum evacuation (unnormalized O + row sums). After both hp units
            # of an iq: batched approx-reciprocal, DMA-broadcast of the recip
            # rows, DVE normalize into OT_sb, then Y projection for iq tokens.
            y_r = y.rearrange("(t p) n -> t p n", p=P)
            for iq in range(NQH):
                q0 = iq * QH
                Ou = []  # unnormalized O^T in SBUF: 4 tiles [64, QH] f32
                Rs = []  # row-sum rows: 4 tiles [1, QH] f32
                for hp in range(MC):
                    O_pair = [
                        psO.tile([D + 1, QH], F32, tag="acc", name=f"O{hp}{iq}a"),
                        psO.tile([D + 1, QH], F32, tag="acc", name=f"O{hp}{iq}b"),
                    ]
                    for kt in range(NT):
                        ST_pair = [
                            psA.tile([P, QH], F32, tag="big", name=f"ST{kt}a"),
                            psA.tile([P, QH], F32, tag="big", name=f"ST{kt}b"),
                        ]
                        # scores^T: row-group packed pair (bases 0 / 64)
                        for qs in range(QH // 512):
                            for i, base in ((0, 0), (1, 64)):
                                nc.tensor.matmul(
                                    ST_pair[i][:, bass.ts(qs, 512)],
                                    lhsT=KT_sb[base : base + 64, hp, bass.ts(kt, P)],
                                    rhs=QT_sb[
                                        base : base + 64,
                                        hp,
                                        q0 + qs * 512 : q0 + (qs + 1) * 512,
                                    ],
                                    start=True,
                                    stop=True,
                                )
                        for i in range(2):
                            h_local = 2 * hp + i
                            PT = work.tile([P, QH], DT, tag="pt")
                            nc.scalar.activation(
                                out=PT, in_=ST_pair[i], func=AF.Exp, scale=SCALE
                            )
                            for qs in range(QH // 512):
                                nc.tensor.matmul(
                                    O_pair[i][:, bass.ts(qs, 512)],
                                    lhsT=V_sb[:, kt, h_local, :],
                                    rhs=PT[:, bass.ts(qs, 512)],
                                    start=(kt == 0),
                                    stop=(kt == NT - 1),
                                )
                    # fast psum evacuation: unnormalized O + row sums
                    for i in range(2):
                        ou = work.tile([64, QH], F32, tag="ou", name=f"ou{hp}{iq}{i}")
                        nc.vector.tensor_copy(out=ou, in_=O_pair[i][0:D, :])
                        rsrow = norm.tile(
                            [1, QH], F32, tag="rs", bufs=4, name=f"rs{hp}{iq}{i}"
                        )
                        nc.vector.tensor_copy(out=rsrow, in_=O_pair[i][D : D + 1, :])
                        Ou.append(ou)
                        Rs.append(rsrow)
                # renorm for all 4 heads of this iq
                rdram = dram.tile([4, QH], F32, tag="rdram", name=f"rd{iq}")
                for u in range(4):
                    rc = norm.tile([1, QH], F32, tag="rc", bufs=4, name=f"rc{iq}{u}")
                    nc.vector.reciprocal_approx_fast(out=rc, in_=Rs[u])
                    nc.sync.dma_start(out=rdram[u : u + 1, :], in_=rc)
                bc = norm.tile([64, 4, QH], F32, tag="bc", name=f"bc{iq}")
                rdram_b = bass.AP(
                    tensor=rdram.tensor,
                    offset=rdram.offset,
                    ap=[[0, 64]] + list(rdram.ap),
                )
                nc.sync.dma_start(out=bc, in_=rdram_b)
                for u, (hp, i) in enumerate(((0, 0), (0, 1), (1, 0), (1, 1))):
                    base = 64 * i
                    nc.vector.tensor_mul(
                        out=OT_sb[base : base + 64, hp, q0 : q0 + QH],
                        in0=Ou[u],
                        in1=bc[:, u, :],
                    )
                # ---- output projection for this iq's tokens ----
                for t in range(iq * (NT // NQH), (iq + 1) * (NT // NQH)):
                    psY = psA.tile([P, E], F32, tag="big", name=f"psY{t}")
                    for n2 in range(E // 512):
                        for mc in range(MC):
                            nc.tensor.matmul(
                                psY[:, bass.ts(n2, 512)],
                                lhsT=OT_sb[:, mc, bass.ts(t, P)],
                                rhs=wo_sb[:, mc, bass.ts(n2, 512)],
                                start=(mc == 0),
                                stop=(mc == MC - 1),
                            )
                    y_sb = outsb.tile([P, E], F32, tag="ysb", name=f"ysb{t}")
                    nc.vector.tensor_copy(out=y_sb, in_=psY)
                    nc.sync.dma_start(out=y_r[t], in_=y_sb)

    nc.compile()
    return nc


_NC_CACHE = {}


def get_nc():
    if "nc" not in _NC_CACHE:
        _NC_CACHE["nc"] = build_nc()
    return _NC_CACHE["nc"]


def make_in_maps(x, Wq, bq, Wk, Wv, Wo):
    xT_by_batch = [
        np.ascontiguousarray(x[b].T).astype(NP_DT) for b in range(B)
    ]
    in_maps = []
    for c in range(NCORES):
        b, hg = divmod(c, NCORES // B)
        hs = slice(hg * EH, (hg + 1) * EH)
        in_maps.append(
            {
                "xT": xT_by_batch[b],
                "wq": np.ascontiguousarray(Wq[:, hs]).astype(NP_DT),
                "wk": np.ascontiguousarray(Wk[:, hs]).astype(NP_DT),
                "wv": np.ascontiguousarray(Wv[:, hs]).astype(NP_DT),
                "wo": np.ascontiguousarray(Wo[hs, :]).astype(NP_DT),
                "bq": np.ascontiguousarray(bq[hs]).astype(np.float32),
            }
        )
    return in_maps


def gather_out(results, bv, Wo, bo):
    bias_row = (
        bv.astype(np.float64) @ Wo.astype(np.float64) + bo.astype(np.float64)
    ).astype(np.float32)
    out = np.empty((B, S, E), np.float32)
    gpb = NCORES // B
    for b in range(B):
        acc = results[gpb * b]["y"].copy()
        for i in range(1, gpb):
            acc += results[gpb * b + i]["y"]
        out[b] = acc + bias_row
    return out


def kernel(x, Wq, bq, Wk, bk, Wv, bv, Wo, bo, **_):
    x = np.asarray(x, np.float32)
    nc = get_nc()
    in_maps = make_in_maps(
        x,
        np.asarray(Wq, np.float32),
        np.asarray(bq, np.float32),
        np.asarray(Wk, np.float32),
        np.asarray(Wv, np.float32),
        np.asarray(Wo, np.float32),
    )
    res = run_bass_kernel_spmd(nc, in_maps, list(range(NCORES)))
    return gather_out(
        res.results, np.asarray(bv, np.float32), np.asarray(Wo, np.float32),
        np.asarray(bo, np.float32)
    )


# revision 12
# speedup vs baseline: 1.3528x; 1.1487x over previous
"""Multi-head self-attention (B=2, S=2048, E=1024, H=16, D=64) on 8 TRN2 cores.

Sharding: tensor-parallel over (batch, head-group): core c handles batch c//4
and heads [4*(c%4), 4*(c%4)+4). Each core computes its heads' attention output
projected through its slice of Wo; the host sums the 4 partial outputs per
batch and adds the constant bias row (bv @ Wo + bo).

Device-side math (per core, transposed formulation so no transposes needed):
  QT = Wq_c^T @ x^T + bq_c        [256, S]   (bias bk dropped: softmax-invariant)
  KT = Wk_c^T @ x^T               [256, S]
  V  = x @ Wv_c                   [S, 256]   (bias bv folded into host bias row)
  S^T tile = K @ Q^T              (PE, per 128-k-token x 1024-q tile)
  P^T = exp(S^T / 8)              (ACT, no max subtraction: scores ~ N(0,1))
  O^T aug = [V | 1]^T @ P^T       (PE, accumulated over k tiles; row 64 = sum)
  O^T = O^T aug[0:64] / row 64    (recip + PE broadcast + DVE mul)
  Y = O @ Wo_c                    [S, 1024]  fp32 partial out
"""

import numpy as np
import ml_dtypes

import concourse.bass as bass
import concourse.bacc as bacc
import concourse.tile as tile
from concourse import mybir
from concourse.bass_utils import run_bass_kernel_spmd

B, S, E = 2, 2048, 1024
H, D = 16, 64
NCORES = 8
HPC = 4                 # heads per core
EH = HPC * D            # 256: per-core head width
P = 128
EC = E // P             # 8 E-chunks of 128
MC = EH // P            # 2 Eh-chunks of 128
NT = S // P             # 16 token tiles of 128
QH = 1024               # q-chunk processed per attention unit
NQH = S // QH           # 2
SCALE = 1.0 / float(np.sqrt(D))

DT = mybir.dt.bfloat16
NP_DT = ml_dtypes.bfloat16
F32 = mybir.dt.float32
F32R = mybir.dt.float32r

AF = mybir.ActivationFunctionType


def build_nc():
    nc = bacc.Bacc(
        "TRN2", target_bir_lowering=False, debug=False, enable_asserts=False
    )
    xT = nc.dram_tensor("xT", [E, S], DT, kind="ExternalInput").ap()
    wq = nc.dram_tensor("wq", [E, EH], DT, kind="ExternalInput").ap()
    wk = nc.dram_tensor("wk", [E, EH], DT, kind="ExternalInput").ap()
    wv = nc.dram_tensor("wv", [E, EH], DT, kind="ExternalInput").ap()
    wo = nc.dram_tensor("wo", [EH, E], DT, kind="ExternalInput").ap()
    bq = nc.dram_tensor("bq", [EH], F32, kind="ExternalInput").ap()
    y = nc.dram_tensor("y", [S, E], F32, kind="ExternalOutput").ap()

    with tile.TileContext(nc) as tc:
        with (
            tc.tile_pool(name="consts", bufs=1) as consts,
            tc.tile_pool(name="work", bufs=4) as work,
            tc.tile_pool(name="norm", bufs=2) as norm,
            tc.tile_pool(name="outsb", bufs=2) as outsb,
            tc.tile_pool(name="psA", bufs=2, space="PSUM") as psA,
            tc.tile_pool(name="psO", bufs=2, space="PSUM") as psO,
            tc.tile_pool(name="dram", bufs=2, space="DRAM") as dram,
        ):
            # ---- constant loads ----
            xT_sb = consts.tile([P, EC, S], DT)
            nc.sync.dma_start(out=xT_sb, in_=xT.rearrange("(c p) s -> p c s", p=P))
            wq_sb = consts.tile([P, EC, EH], DT)
            nc.sync.dma_start(out=wq_sb, in_=wq.rearrange("(c p) n -> p c n", p=P))
            wk_sb = consts.tile([P, EC, EH], DT)
            nc.sync.dma_start(out=wk_sb, in_=wk.rearrange("(c p) n -> p c n", p=P))
            wv_sb = consts.tile([P, EC, EH], DT)
            nc.sync.dma_start(out=wv_sb, in_=wv.rearrange("(c p) n -> p c n", p=P))
            wo_sb = consts.tile([P, MC, E], DT)
            nc.sync.dma_start(out=wo_sb, in_=wo.rearrange("(m p) n -> p m n", p=P))
            bq_sb = consts.tile([P, MC], F32)
            nc.sync.dma_start(out=bq_sb, in_=bq.rearrange("(m p) -> p m", p=P))


            QT_sb = consts.tile([P, MC, S], DT)
            KT_sb = consts.tile([P, MC, S], DT)
            V_sb = consts.tile([P, NT, HPC, D + 1], DT)
            OT_sb = consts.tile([P, MC, S], DT)
            nc.vector.memset(V_sb[:, :, :, D : D + 1], 1.0)

            # ---- QKV projections (interleaved for engine balance) ----
            # Per 512-token chunk t4: Q and K chunks (ACT evac) then 4 V token
            # tiles (DVE evac) so PE / ACT / DVE all stay busy.
            for t4 in range(S // 512):
                sl = bass.ts(t4, 512)
                for mc in range(MC):
                    for w_sb, dst, is_q in (
                        (wq_sb, QT_sb, True),
                        (wk_sb, KT_sb, False),
                    ):
                        ps = psA.tile([P, 512], F32, tag="big", name=f"qk{t4}{mc}{is_q}")
                        for ec in range(EC):
                            nc.tensor.matmul(
                                ps,
                                lhsT=w_sb[:, ec, mc * P : (mc + 1) * P],
                                rhs=xT_sb[:, ec, sl],
                                start=(ec == 0),
                                stop=(ec == EC - 1),
                            )
                        if is_q:
                            nc.scalar.activation(
                                out=dst[:, mc, sl],
                                in_=ps,
                                func=AF.Identity,
                                bias=bq_sb[:, mc : mc + 1],
                                scale=1.0,
                            )
                        else:
                            nc.scalar.copy(out=dst[:, mc, sl], in_=ps)
                for t in range(4 * t4, 4 * t4 + 4):
                    ps = psA.tile([P, EH], F32, tag="big", name=f"v{t}")
                    for ec in range(EC):
                        nc.tensor.matmul(
                            ps,
                            lhsT=xT_sb[:, ec, bass.ts(t, P)],
                            rhs=wv_sb[:, ec, :],
                            start=(ec == 0),
                            stop=(ec == EC - 1),
                        )
                    nc.vector.tensor_copy(
                        out=V_sb[:, t, :, 0:D],
                        in_=ps.rearrange("p (h d) -> p h d", h=HPC),
                    )

            # ---- attention + output projection, software pipelined ----
            y_r = y.rearrange("(t p) n -> t p n", p=P)

            def att_unit(hp, iq, Ou, Rs):
                """Scores^T -> exp -> [V|1]^T @ P^T for heads (2hp, 2hp+1) on
                q-chunk iq; evacuates unnormalized O^T + row sums to SBUF."""
                q0 = iq * QH
                O_pair = [
                    psO.tile([D + 1, QH], F32, tag="acc", name=f"O{hp}{iq}a"),
                    psO.tile([D + 1, QH], F32, tag="acc", name=f"O{hp}{iq}b"),
                ]
                for kt in range(NT):
                    ST_pair = [
                        psA.tile([P, QH], F32, tag="big", name=f"ST{hp}{iq}{kt}a"),
                        psA.tile([P, QH], F32, tag="big", name=f"ST{hp}{iq}{kt}b"),
                    ]
                    # scores^T: row-group packed pair (bases 0 / 64)
                    for qs in range(QH // 512):
                        for i, base in ((0, 0), (1, 64)):
                            nc.tensor.matmul(
                                ST_pair[i][:, bass.ts(qs, 512)],
                                lhsT=KT_sb[base : base + 64, hp, bass.ts(kt, P)],
                                rhs=QT_sb[
                                    base : base + 64,
                                    hp,
                                    q0 + qs * 512 : q0 + (qs + 1) * 512,
                                ],
                                start=True,
                                stop=True,
                            )
                    for i in range(2):
                        h_local = 2 * hp + i
                        PT = work.tile([P, QH], DT, tag="pt", name=f"PT{hp}{iq}{kt}{i}")
                        nc.scalar.activation(
                            out=PT, in_=ST_pair[i], func=AF.Exp, scale=SCALE
                        )
                        for qs in range(QH // 512):
                            nc.tensor.matmul(
                                O_pair[i][:, bass.ts(qs, 512)],
                                lhsT=V_sb[:, kt, h_local, :],
                                rhs=PT[:, bass.ts(qs, 512)],
                                start=(kt == 0),
                                stop=(kt == NT - 1),
                            )
                # fast psum evacuation: unnormalized O + row sums
                for i in range(2):
                    ou = work.tile([64, QH], F32, tag="ou", name=f"ou{hp}{iq}{i}")
                    nc.vector.tensor_copy(out=ou, in_=O_pair[i][0:D, :])
                    rsrow = norm.tile(
                        [1, QH], F32, tag="rs", bufs=4, name=f"rs{hp}{iq}{i}"
                    )
                    nc.vector.tensor_copy(out=rsrow, in_=O_pair[i][D : D + 1, :])
                    Ou.append(ou)
                    Rs.append(rsrow)

            def normalize(iq, Ou, Rs):
                """Batched approx-recip + DMA broadcast + DVE renorm into OT_sb."""
                q0 = iq * QH
                rdram = dram.tile([4, QH], F32, tag="rdram", name=f"rd{iq}")
                for u in range(4):
                    rc = norm.tile([1, QH], F32, tag="rc", bufs=4, name=f"rc{iq}{u}")
                    nc.vector.reciprocal_approx_fast(out=rc, in_=Rs[u])
                    nc.sync.dma_start(out=rdram[u : u + 1, :], in_=rc)
                bc = norm.tile([64, 4, QH], F32, tag="bc", name=f"bc{iq}")
                rdram_b = bass.AP(
                    tensor=rdram.tensor,
                    offset=rdram.offset,
                    ap=[[0, 64]] + list(rdram.ap),
                )
                nc.sync.dma_start(out=bc, in_=rdram_b)
                for u, (hp, i) in enumerate(((0, 0), (0, 1), (1, 0), (1, 1))):
                    nc.vector.tensor_mul(
                        out=OT_sb[64 * i : 64 * i + 64, hp, q0 : q0 + QH],
                        in0=Ou[u],
                        in1=bc[:, u, :],
                    )

            def y_proj(iq):
                for t in range(iq * (NT // NQH), (iq + 1) * (NT // NQH)):
                    psY = psA.tile([P, E], F32, tag="big", name=f"psY{t}")
                    for n2 in range(E // 512):
                        for mc in range(MC):
                            nc.tensor.matmul(
                                psY[:, bass.ts(n2, 512)],
                                lhsT=OT_sb[:, mc, bass.ts(t, P)],
                                rhs=wo_sb[:, mc, bass.ts(n2, 512)],
                                start=(mc == 0),
                                stop=(mc == MC - 1),
                            )
                    y_sb = outsb.tile([P, E], F32, tag="ysb", name=f"ysb{t}")
                    nc.vector.tensor_copy(out=y_sb, in_=psY)
                    nc.sync.dma_start(out=y_r[t], in_=y_sb)

            # Pipeline: Y(iq) is emitted after att(0, iq+1) so the PE has a
            # full unit of attention work queued before it reaches Y's
            # dependency on the normalize chain (engines run in order).
            state = {}
            for iq in range(NQH):
                Ou, Rs = [], []
                for hp in range(MC):
                    att_unit(hp, iq, Ou, Rs)
                    if hp == 0 and iq > 0:
                        y_proj(iq - 1)
                normalize(iq, Ou, Rs)
            y_proj(NQH - 1)

    nc.compile()
    return nc


_NC_CACHE = {}


def get_nc():
    if "nc" not in _NC_CACHE:
        _NC_CACHE["nc"] = build_nc()
    return _NC_CACHE["nc"]


def make_in_maps(x, Wq, bq, Wk, Wv, Wo):
    xT_by_batch = [
        np.ascontiguousarray(x[b].T).astype(NP_DT) for b in range(B)
    ]
    in_maps = []
    for c in range(NCORES):
        b, hg = divmod(c, NCORES // B)
        hs = slice(hg * EH, (hg + 1) * EH)
        in_maps.append(
            {
                "xT": xT_by_batch[b],
                "wq": np.ascontiguousarray(Wq[:, hs]).astype(NP_DT),
                "wk": np.ascontiguousarray(Wk[:, hs]).astype(NP_DT),
                "wv": np.ascontiguousarray(Wv[:, hs]).astype(NP_DT),
                "wo": np.ascontiguousarray(Wo[hs, :]).astype(NP_DT),
                "bq": np.ascontiguousarray(bq[hs]).astype(np.float32),
            }
        )
    return in_maps


def gather_out(results, bv, Wo, bo):
    bias_row = (
        bv.astype(np.float64) @ Wo.astype(np.float64) + bo.astype(np.float64)
    ).astype(np.float32)
    out = np.empty((B, S, E), np.float32)
    gpb = NCORES // B
    for b in range(B):
        acc = results[gpb * b]["y"].copy()
        for i in range(1, gpb):
            acc += results[gpb * b + i]["y"]
        out[b] = acc + bias_row
    return out


def kernel(x, Wq, bq, Wk, bk, Wv, bv, Wo, bo, **_):
    x = np.asarray(x, np.float32)
    nc = get_nc()
    in_maps = make_in_maps(
        x,
        np.asarray(Wq, np.float32),
        np.asarray(bq, np.float32),
        np.asarray(Wk, np.float32),
        np.asarray(Wv, np.float32),
        np.asarray(Wo, np.float32),
    )
    res = run_bass_kernel_spmd(nc, in_maps, list(range(NCORES)))
    return gather_out(
        res.results, np.asarray(bv, np.float32), np.asarray(Wo, np.float32),
        np.asarray(bo, np.float32)
    )
